# revision 25
# baseline (speedup 1.0000x reference)
"""Trainium2 Bass kernel for 16-head MHA (B=2, N=2048, D=1024, H=16).

Sharding: 8 cores = batch(2) x head-group(4). Each core computes 4 heads of
one batch element end-to-end (QKV projections, attention, and its partial
contribution to the output projection). The output projection is a sum over
head features, so each core returns a (N, D) partial product; the host sums
the 4 partials per batch and adds the output bias during unshard.

Per-core dataflow (all matmuls bf16 inputs, f32 PSUM accumulation):
  qT/kT = W @ x.T          (head-feature-major layout, 2 tiles of (128, N))
  v     = x @ Wv.T + bv    stored per key-tile as [v_h | ones] blocks
  scoresT[kt] = k @ q.T    (keys on partitions -> softmax denom comes from PE)
  expT = exp(SCALE*scoresT)  on ScalarE, reading PSUM directly
  [yT; denom] += [v|1].T @ expT  accumulated over key tiles
  yT_norm = yT * 1/denom   (denominator broadcast across partitions by PE)
  partial = yT_norm.T @ WoT
Heads are processed in pairs occupying partitions 0-63 / 64-127 so the two
scoresT matmuls (contraction K=64) row-pack onto disjoint PE row groups.
"""

import sys
from contextlib import ExitStack

import numpy as np

if "/opt/trn_rl_repo" not in sys.path:
    sys.path.insert(0, "/opt/trn_rl_repo")

import ml_dtypes

P = 128
B = 2
NTOK = 2048  # sequence length
D = 1024  # model dim
H_PER_CORE = 4  # heads per core
HD = 64  # head dim
DG = H_PER_CORE * HD  # head-group feature width per core (256)
QB = 512  # query block (matmul free dim)
N_QB = NTOK // QB  # 4
N_KT = NTOK // P  # 16 key tiles
N_DT = D // P  # 8 contraction tiles for projections
SCALE = HD ** -0.5

_BF16 = ml_dtypes.bfloat16


def _emit(tc, t):
    import concourse.bass as bass
    from concourse import mybir

    F32 = mybir.dt.float32
    BF16 = mybir.dt.bfloat16
    Exp = mybir.ActivationFunctionType.Exp
    nc = tc.nc

    with ExitStack() as ctx:
        consts = ctx.enter_context(tc.tile_pool(name="consts", bufs=1))
        # PSUM budget (8 banks): sc 2x2 + pv 2x1 + pp 2x1 = 8
        pp_psum = ctx.enter_context(
            tc.tile_pool(name="pp_psum", bufs=2, space="PSUM")
        )
        sc_psum = ctx.enter_context(
            tc.tile_pool(name="sc_psum", bufs=2, space="PSUM")
        )
        pv_psum = ctx.enter_context(
            tc.tile_pool(name="pv_psum", bufs=1, space="PSUM")
        )
        expT_pool = ctx.enter_context(tc.tile_pool(name="expT", bufs=18))
        nrm_pool = ctx.enter_context(tc.tile_pool(name="nrm", bufs=4))
        ob_pool = ctx.enter_context(tc.tile_pool(name="ob", bufs=4))

        # ---- resident SBUF tensors (single tiles, one DMA each) ----
        xT_t = consts.tile([P, N_DT * NTOK], BF16, tag="xT", name="xT")
        wq_t = consts.tile([P, N_DT * DG], BF16, tag="wq", name="wq")
        wk_t = consts.tile([P, N_DT * DG], BF16, tag="wk", name="wk")
        wv_t = consts.tile([P, N_DT * DG], BF16, tag="wv", name="wv")
        wo_t = consts.tile([P, (DG // P) * D], BF16, tag="wo", name="wo")
        bq_t = consts.tile([P, DG // P], F32, tag="bq", name="bq")
        bk_t = consts.tile([P, DG // P], F32, tag="bk", name="bk")
        bvb_t = consts.tile([P, DG], F32, tag="bvb", name="bvb")
        qT_t = [consts.tile([P, NTOK], BF16, tag=f"qT{i}", name=f"qT{i}") for i in range(DG // P)]
        kT_t = [consts.tile([P, NTOK], BF16, tag=f"kT{i}", name=f"kT{i}") for i in range(DG // P)]
        # v per key tile: 4 head blocks of [v_h (64 cols) | ones (64 cols)]
        v_t = [
            consts.tile([P, H_PER_CORE * 2 * HD], BF16, tag=f"v{i}", name=f"v{i}")
            for i in range(N_KT)
        ]
        yT_t = [consts.tile([P, NTOK], BF16, tag=f"yT{i}", name=f"yT{i}") for i in range(DG // P)]

        # ---- ACT exp-table warmup (hides ~2.7us ACT_TABLE_LOAD) ----
        warm = nrm_pool.tile([P, 32], F32, tag="warm", name="warm", bufs=1)
        nc.vector.memset(warm[:], 0.0)
        nc.scalar.activation(warm[:, 0:16], warm[:, 16:32], Exp)

        # ---- input DMAs: few big descriptors, ordered so the first
        # projection group (k pair0 qb0) can start ~4us in ----
        xT_v = xT_t.rearrange("p (a n) -> p a n", a=N_DT)
        xT_d = t["xT"].rearrange("(a p) n -> p a n", p=P)

        def dma_x(qb):
            qsl = slice(qb * QB, (qb + 1) * QB)
            nc.sync.dma_start(xT_v[:, :, qsl], xT_d[:, :, qsl])

        def dma_w(dst, key, a):
            nc.sync.dma_start(
                dst.rearrange("p (a d) -> p a d", a=a),
                t[key].rearrange("(a p) d -> p a d", p=P),
            )

        dma_w(wk_t, "wkT", N_DT)
        dma_x(0)
        dma_w(wq_t, "wqT", N_DT)
        nc.sync.dma_start(bk_t[:], t["bk2"][:])
        nc.sync.dma_start(bq_t[:], t["bq2"][:])
        dma_x(1)
        dma_w(wv_t, "wvT", N_DT)
        nc.sync.dma_start(bvb_t[:], t["bvb"][:])
        dma_x(2)
        dma_x(3)
        dma_w(wo_t, "woT", DG // P)


        # slice helpers into the packed single tiles
        def w_sl(wt, dt, pt):
            o = dt * DG + pt * P
            return wt[:, o : o + P]

        def x_sl(dt, lo, hi):
            return xT_t[:, dt * NTOK + lo : dt * NTOK + hi]

        # ---- projection group emitters (each: 8 matmuls + eviction) ----
        def qk_group(wt, bt, dst, pt, qb):
            pp = pp_psum.tile([P, QB], F32, tag="pp", name="pp")
            for dt in range(N_DT):
                nc.tensor.matmul(
                    pp[:],
                    lhsT=w_sl(wt, dt, pt),
                    rhs=x_sl(dt, qb * QB, (qb + 1) * QB),
                    start=(dt == 0),
                    stop=(dt == N_DT - 1),
                )
            nc.vector.tensor_scalar_add(
                dst[pt][:, qb * QB : (qb + 1) * QB], pp[:], bt[:, pt : pt + 1]
            )

        def v_group(kt):
            pp = pp_psum.tile([P, QB], F32, tag="pp", name="pp_v")
            for dt in range(N_DT):
                nc.tensor.matmul(
                    pp[:, 0:DG],
                    lhsT=x_sl(dt, kt * P, (kt + 1) * P),
                    rhs=wv_t[:, dt * DG : (dt + 1) * DG],
                    start=(dt == 0),
                    stop=(dt == N_DT - 1),
                )
            vk = v_t[kt].rearrange("p (h w) -> p h w", h=H_PER_CORE)
            nc.vector.tensor_add(
                vk[:, :, 0:HD],
                pp[:, 0:DG].rearrange("p (h w) -> p h w", h=H_PER_CORE),
                bvb_t[:].rearrange("p (h w) -> p h w", h=H_PER_CORE),
            )
            nc.vector.memset(vk[:, :, HD : 2 * HD], 1.0)

        # ---- PV + normalize emitters (operate on the PREVIOUS body's
        # exp tiles, so the PE never waits on ScalarE) ----
        def pv_alloc():
            return [
                pv_psum.tile([P, QB], F32, tag="pv_a", name="pv_a"),
                pv_psum.tile([P, QB], F32, tag="pv_b", name="pv_b"),
            ]

        def pv_step(state, kt):
            pair, qb, pv, ex_list = state
            for h2 in (0, 1):
                head = 2 * pair + h2
                nc.tensor.matmul(
                    pv[h2][:],
                    lhsT=v_t[kt][:, head * 2 * HD : (head + 1) * 2 * HD],
                    rhs=ex_list[kt][:, h2 * QB : (h2 + 1) * QB],
                    start=(kt == 0),
                    stop=(kt == N_KT - 1),
                )

        def norm(state):
            pair, qb, pv, ex_list = state
            qsl = slice(qb * QB, (qb + 1) * QB)
            for h2 in (0, 1):
                cpy = nrm_pool.tile([HD, QB], F32, tag="cpy", name="cpy")
                cpd = nrm_pool.tile([HD, QB], F32, tag="cpd", name="cpd")
                nc.vector.tensor_copy(cpy[:], pv[h2][0:HD, :])
                nc.vector.tensor_copy(cpd[:], pv[h2][HD:P, :])
                dst = yT_t[pair][h2 * HD : (h2 + 1) * HD, qsl]
                rc = nrm_pool.tile([HD, QB], F32, tag="rc", name="rc")
                # approx_fast is ~18-bit accurate; only valid with
                # base-partition-0 operands (breaks at other bases on HW)
                nc.vector.reciprocal_approx_fast(rc[:], cpd[:])
                nc.vector.tensor_mul(dst, cpy[:], rc[:])

        # ---- output projection for one token tile (128 rows) ----
        def oproj_mt(mt, evict_engine=None):
            msl = slice(mt * P, (mt + 1) * P)
            ob = ob_pool.tile([P, D], F32, tag="ob", name="ob")
            for nb in range(D // QB):
                op = pp_psum.tile([P, QB], F32, tag="pp", name="op")
                for ktile in range(DG // P):
                    nc.tensor.matmul(
                        op[:],
                        lhsT=yT_t[ktile][:, msl],
                        rhs=wo_t[:, ktile * D + nb * QB : ktile * D + (nb + 1) * QB],
                        start=(ktile == 0),
                        stop=(ktile == DG // P - 1),
                    )
                # tail tiles split evictions across ScalarE/VectorE
                if evict_engine and nb % 2 == 0:
                    nc.scalar.copy(ob[:, nb * QB : (nb + 1) * QB], op[:])
                else:
                    nc.vector.tensor_copy(ob[:, nb * QB : (nb + 1) * QB], op[:])
            nc.sync.dma_start(t["partial"][msl, :], ob[:])

        # ---- attention body: emits SC+exp for (pair, qb) while draining
        # the previous body's PV accumulation (2 steps/chunk over chunks
        # 0-7), its normalize at chunk 8, and filler work. Returns a state
        # tuple the next body uses to finish this iteration. ----
        def body(pair, qb, prev_state, fillers={}, self_pv=False, drain_rate=2):
            kt_p = kT_t[pair]
            qt_p = qT_t[pair]
            qsl = slice(qb * QB, (qb + 1) * QB)
            ex_list = []
            pv = None
            pv_self = [None]
            for kt in range(N_KT):
                ksl = slice(kt * P, (kt + 1) * P)
                sc = sc_psum.tile([P, 2 * QB], F32, tag="sc", name="sc")
                # head-pair scoresT matmuls (K=64) row-pack onto disjoint
                # PE row groups and run concurrently
                nc.tensor.matmul(
                    sc[:, 0:QB],
                    lhsT=kt_p[0:HD, ksl],
                    rhs=qt_p[0:HD, qsl],
                    start=True,
                    stop=True,
                )
                nc.tensor.matmul(
                    sc[:, QB : 2 * QB],
                    lhsT=kt_p[HD:P, ksl],
                    rhs=qt_p[HD:P, qsl],
                    start=True,
                    stop=True,
                )
                ex = expT_pool.tile([P, 2 * QB], BF16, tag="ex", name="ex")
                nc.scalar.activation(ex[:], sc[:], Exp, scale=SCALE)
                ex_list.append(ex)
                if prev_state is not None:
                    if pv is None:
                        pv = pv_alloc()
                        prev_state = (prev_state[0], prev_state[1], pv, prev_state[2])
                    if drain_rate == 1:
                        pv_step(prev_state, kt)
                    elif kt < 8:
                        pv_step(prev_state, 2 * kt)
                        pv_step(prev_state, 2 * kt + 1)
                    elif kt == 8:
                        norm(prev_state)
                    elif self_pv and kt >= 9:
                        # last body: start own PV on freed slots (chunks 9-15)
                        if pv_self[0] is None:
                            pv_self[0] = pv_alloc()
                        st = (pair, qb, pv_self[0], ex_list)
                        pv_step(st, 2 * (kt - 9))
                        pv_step(st, 2 * (kt - 9) + 1)
                for f in fillers.get(kt, ()):
                    f()
            if prev_state is not None and drain_rate == 1:
                norm(prev_state)
            if self_pv:
                st = (pair, qb, pv_self[0], ex_list)
                pv_step(st, 14)
                pv_step(st, 15)
                norm(st)
                return None
            return (pair, qb, ex_list)

        def drain(prev_state):
            if prev_state is None:
                return
            pv = pv_alloc()
            st = (prev_state[0], prev_state[1], pv, prev_state[2])
            for kt in range(N_KT):
                pv_step(st, kt)
            norm(st)

        # ---- schedule ----
        G = lambda *a: (lambda: qk_group(*a))
        V = lambda kt: (lambda: v_group(kt))
        O = lambda mt, eng=None: (lambda: oproj_mt(mt, eng))

        qk_group(wk_t, bk_t, kT_t, 0, 0)
        qk_group(wq_t, bq_t, qT_t, 0, 0)
        s = body(0, 0, None, fillers={
            2: [G(wk_t, bk_t, kT_t, 0, 1)],
            5: [G(wk_t, bk_t, kT_t, 0, 2)],
            8: [G(wk_t, bk_t, kT_t, 0, 3)],
            3: [V(0)], 4: [V(1)], 6: [V(2)], 7: [V(3)],
            9: [V(4)], 10: [V(5)], 11: [V(6)], 12: [V(7)],
            13: [G(wq_t, bq_t, qT_t, 0, 1)],
            14: [G(wq_t, bq_t, qT_t, 0, 2)],
            15: [G(wq_t, bq_t, qT_t, 0, 3)],
        })
        s = body(0, 1, s, drain_rate=1, fillers={
            **{kt: [V(kt + 8)] for kt in range(8)},
            9: [G(wk_t, bk_t, kT_t, 1, 0)],
            12: [G(wk_t, bk_t, kT_t, 1, 1)],
        })
        s = body(0, 2, s, fillers={
            9: [G(wk_t, bk_t, kT_t, 1, 2)],
            11: [G(wk_t, bk_t, kT_t, 1, 3)],
            13: [G(wq_t, bq_t, qT_t, 1, 0)],
        })
        s = body(0, 3, s, fillers={
            9: [G(wq_t, bq_t, qT_t, 1, 1)],
            11: [G(wq_t, bq_t, qT_t, 1, 2)],
            13: [G(wq_t, bq_t, qT_t, 1, 3)],
        })
        s = body(1, 0, s)
        s = body(1, 1, s, fillers={
            9: [O(0)], 11: [O(1)], 13: [O(2)], 15: [O(3)],
        })
        s = body(1, 2, s, fillers={
            9: [O(4)], 11: [O(5)], 13: [O(6)], 15: [O(7)],
        })
        s = body(1, 3, s, self_pv=True, fillers={
            9: [O(8)], 11: [O(9)], 13: [O(10)], 15: [O(11)],
        })
        drain(s)
        for mt in range(12, 16):
            oproj_mt(mt, nc.scalar)


def _build():
    import concourse.bacc as bacc
    import concourse.tile as tile
    from concourse import mybir

    F32 = mybir.dt.float32
    BF16 = mybir.dt.bfloat16

    nc = bacc.Bacc(
        "TRN2", target_bir_lowering=False, debug=False, num_devices=8
    )
    t = {
        "xT": nc.dram_tensor("xT", (D, NTOK), BF16, kind="ExternalInput").ap(),
        "wqT": nc.dram_tensor("wqT", (D, DG), BF16, kind="ExternalInput").ap(),
        "wkT": nc.dram_tensor("wkT", (D, DG), BF16, kind="ExternalInput").ap(),
        "wvT": nc.dram_tensor("wvT", (D, DG), BF16, kind="ExternalInput").ap(),
        "woT": nc.dram_tensor("woT", (DG, D), BF16, kind="ExternalInput").ap(),
        "bq2": nc.dram_tensor(
            "bq2", (P, DG // P), F32, kind="ExternalInput"
        ).ap(),
        "bk2": nc.dram_tensor(
            "bk2", (P, DG // P), F32, kind="ExternalInput"
        ).ap(),
        "bvb": nc.dram_tensor("bvb", (P, DG), F32, kind="ExternalInput").ap(),
        "partial": nc.dram_tensor(
            "partial", (NTOK, D), F32, kind="ExternalOutput"
        ).ap(),
    }
    with tile.TileContext(nc) as tc:
        _emit(tc, t)
    nc.compile()
    return nc


_CACHE = {}


def _get_nc():
    if "nc" not in _CACHE:
        _CACHE["nc"] = _build()
    return _CACHE["nc"]


def make_in_maps(x, Wq, bq, Wk, bk, Wv, bv, Wo):
    """Per-core host-side sharding: core c -> batch c//4, head group c%4."""
    in_maps = []
    for c in range(8):
        b, g = divmod(c, 4)
        sl = slice(DG * g, DG * (g + 1))
        in_maps.append(
            {
                "xT": np.ascontiguousarray(x[b].T).astype(_BF16),
                "wqT": np.ascontiguousarray(Wq[sl].T).astype(_BF16),
                "wkT": np.ascontiguousarray(Wk[sl].T).astype(_BF16),
                "wvT": np.ascontiguousarray(Wv[sl].T).astype(_BF16),
                "woT": np.ascontiguousarray(Wo[:, sl].T).astype(_BF16),
                "bq2": np.ascontiguousarray(
                    bq[sl].reshape(DG // P, P).T
                ).astype(np.float32),
                "bk2": np.ascontiguousarray(
                    bk[sl].reshape(DG // P, P).T
                ).astype(np.float32),
                "bvb": np.ascontiguousarray(
                    np.broadcast_to(bv[sl][None, :], (P, DG))
                ).astype(np.float32),
            }
        )
    return in_maps


def kernel(x, Wq, bq, Wk, bk, Wv, bv, Wo, bo, _spmd_kwargs=None):
    from concourse.bass_utils import run_bass_kernel_spmd

    x, Wq, bq, Wk, bk, Wv, bv, Wo, bo = (
        np.asarray(a, np.float32) for a in (x, Wq, bq, Wk, bk, Wv, bv, Wo, bo)
    )
    nc = _get_nc()
    in_maps = make_in_maps(x, Wq, bq, Wk, bk, Wv, bv, Wo)
    res = run_bass_kernel_spmd(
        nc, in_maps, list(range(8)), **(_spmd_kwargs or {})
    )
    _CACHE["last_results"] = res
    out = np.empty((B, NTOK, D), np.float32)
    for b in range(B):
        acc = res.results[4 * b]["partial"].astype(np.float32).copy()
        for g in range(1, 4):
            acc += res.results[4 * b + g]["partial"]
        out[b] = acc + bo[None, :]
    return out


# revision 27
# speedup vs baseline: 1.0107x; 1.0107x over previous
"""Trainium2 Bass kernel for 16-head MHA (B=2, N=2048, D=1024, H=16).

Sharding: 8 cores = batch(2) x head-group(4). Each core computes 4 heads of
one batch element end-to-end (QKV projections, attention, and its partial
contribution to the output projection). The output projection is a sum over
head features, so each core returns a (N, D) partial product; the host sums
the 4 partials per batch and adds the output bias during unshard.

Per-core dataflow (all matmuls bf16 inputs, f32 PSUM accumulation):
  qT/kT = W @ x.T          (head-feature-major layout, 2 tiles of (128, N))
  v     = x @ Wv.T + bv    stored per key-tile as [v_h | ones] blocks
  scoresT[kt] = k @ q.T    (keys on partitions -> softmax denom comes from PE)
  expT = exp(SCALE*scoresT)  on ScalarE, reading PSUM directly
  [yT; denom] += [v|1].T @ expT  accumulated over key tiles
  yT_norm = yT * 1/denom   (denominator broadcast across partitions by PE)
  partial = yT_norm.T @ WoT
Heads are processed in pairs occupying partitions 0-63 / 64-127 so the two
scoresT matmuls (contraction K=64) row-pack onto disjoint PE row groups.
"""

import sys
from contextlib import ExitStack

import numpy as np

if "/opt/trn_rl_repo" not in sys.path:
    sys.path.insert(0, "/opt/trn_rl_repo")

import ml_dtypes

P = 128
B = 2
NTOK = 2048  # sequence length
D = 1024  # model dim
H_PER_CORE = 4  # heads per core
HD = 64  # head dim
DG = H_PER_CORE * HD  # head-group feature width per core (256)
QB = 512  # query block (matmul free dim)
N_QB = NTOK // QB  # 4
N_KT = NTOK // P  # 16 key tiles
N_DT = D // P  # 8 contraction tiles for projections
SCALE = HD ** -0.5

_BF16 = ml_dtypes.bfloat16


def _emit(tc, t):
    import concourse.bass as bass
    from concourse import mybir

    F32 = mybir.dt.float32
    BF16 = mybir.dt.bfloat16
    Exp = mybir.ActivationFunctionType.Exp
    nc = tc.nc

    with ExitStack() as ctx:
        consts = ctx.enter_context(tc.tile_pool(name="consts", bufs=1))
        # PSUM budget (8 banks): sc 2x2 + pv 2x1 + pp 2x1 = 8
        pp_psum = ctx.enter_context(
            tc.tile_pool(name="pp_psum", bufs=2, space="PSUM")
        )
        sc_psum = ctx.enter_context(
            tc.tile_pool(name="sc_psum", bufs=2, space="PSUM")
        )
        pv_psum = ctx.enter_context(
            tc.tile_pool(name="pv_psum", bufs=1, space="PSUM")
        )
        expT_pool = ctx.enter_context(tc.tile_pool(name="expT", bufs=18))
        nrm_pool = ctx.enter_context(tc.tile_pool(name="nrm", bufs=4))
        ob_pool = ctx.enter_context(tc.tile_pool(name="ob", bufs=4))

        # ---- resident SBUF tensors (single tiles, one DMA each) ----
        xT_t = consts.tile([P, N_DT * NTOK], BF16, tag="xT", name="xT")
        wq_t = consts.tile([P, N_DT * DG], BF16, tag="wq", name="wq")
        wk_t = consts.tile([P, N_DT * DG], BF16, tag="wk", name="wk")
        wv_t = consts.tile([P, N_DT * DG], BF16, tag="wv", name="wv")
        wo_t = consts.tile([P, (DG // P) * D], BF16, tag="wo", name="wo")
        bq_t = consts.tile([P, DG // P], F32, tag="bq", name="bq")
        bk_t = consts.tile([P, DG // P], F32, tag="bk", name="bk")
        bvb_t = consts.tile([P, DG], F32, tag="bvb", name="bvb")
        qT_t = [consts.tile([P, NTOK], BF16, tag=f"qT{i}", name=f"qT{i}") for i in range(DG // P)]
        kT_t = [consts.tile([P, NTOK], BF16, tag=f"kT{i}", name=f"kT{i}") for i in range(DG // P)]
        # v per key tile: 4 head blocks of [v_h (64 cols) | ones (64 cols)]
        v_t = [
            consts.tile([P, H_PER_CORE * 2 * HD], BF16, tag=f"v{i}", name=f"v{i}")
            for i in range(N_KT)
        ]
        yT_t = [consts.tile([P, NTOK], BF16, tag=f"yT{i}", name=f"yT{i}") for i in range(DG // P)]

        # ---- ACT exp-table warmup (hides ~2.7us ACT_TABLE_LOAD) ----
        warm = nrm_pool.tile([P, 32], F32, tag="warm", name="warm", bufs=1)
        nc.vector.memset(warm[:], 0.0)
        nc.scalar.activation(warm[:, 0:16], warm[:, 16:32], Exp)

        # ---- input DMAs: host pre-packs every tensor into its exact SBUF
        # layout, so each load is a fully contiguous 2D burst ----
        xT_v = xT_t.rearrange("p (a n) -> p a n", a=N_DT)

        def dma_x(qb):
            qsl = slice(qb * QB, (qb + 1) * QB)
            nc.sync.dma_start(
                xT_v[:, :, qsl],
                t["xq"][qb].rearrange("p (a n) -> p a n", a=N_DT),
            )

        nc.sync.dma_start(wk_t[:], t["wkT"][:])
        dma_x(0)
        nc.sync.dma_start(wq_t[:], t["wqT"][:])
        nc.sync.dma_start(bk_t[:], t["bk2"][:])
        nc.sync.dma_start(bq_t[:], t["bq2"][:])
        dma_x(1)
        nc.sync.dma_start(wv_t[:], t["wvT"][:])
        nc.sync.dma_start(bvb_t[:], t["bvb"][:])
        dma_x(2)
        dma_x(3)
        nc.sync.dma_start(wo_t[:], t["woT"][:])

        # ---- PE HAM warmup: dummy matmuls fill the DMA wait so the real
        # projections start at the 2.4GHz clock ----
        wmm = nrm_pool.tile([P, 16], BF16, tag="wmm", name="wmm", bufs=1)
        nc.vector.memset(wmm[:], 0.5)
        wps = pp_psum.tile([P, 16], F32, tag="pp", name="wps")
        for _ in range(24):
            nc.tensor.matmul(
                wps[0:16, :], lhsT=wmm[:, 0:16], rhs=wmm[:], start=True, stop=True
            )
        nc.vector.tensor_copy(warm[0:16, 16:32], wps[0:16, :])


        # slice helpers into the packed single tiles
        def w_sl(wt, dt, pt):
            o = dt * DG + pt * P
            return wt[:, o : o + P]

        def x_sl(dt, lo, hi):
            return xT_t[:, dt * NTOK + lo : dt * NTOK + hi]

        # ---- projection group emitters (each: 8 matmuls + eviction) ----
        def qk_group(wt, bt, dst, pt, qb):
            pp = pp_psum.tile([P, QB], F32, tag="pp", name="pp")
            for dt in range(N_DT):
                nc.tensor.matmul(
                    pp[:],
                    lhsT=w_sl(wt, dt, pt),
                    rhs=x_sl(dt, qb * QB, (qb + 1) * QB),
                    start=(dt == 0),
                    stop=(dt == N_DT - 1),
                )
            nc.vector.tensor_scalar_add(
                dst[pt][:, qb * QB : (qb + 1) * QB], pp[:], bt[:, pt : pt + 1]
            )

        def v_group(kt):
            pp = pp_psum.tile([P, QB], F32, tag="pp", name="pp_v")
            for dt in range(N_DT):
                nc.tensor.matmul(
                    pp[:, 0:DG],
                    lhsT=x_sl(dt, kt * P, (kt + 1) * P),
                    rhs=wv_t[:, dt * DG : (dt + 1) * DG],
                    start=(dt == 0),
                    stop=(dt == N_DT - 1),
                )
            vk = v_t[kt].rearrange("p (h w) -> p h w", h=H_PER_CORE)
            nc.vector.tensor_add(
                vk[:, :, 0:HD],
                pp[:, 0:DG].rearrange("p (h w) -> p h w", h=H_PER_CORE),
                bvb_t[:].rearrange("p (h w) -> p h w", h=H_PER_CORE),
            )
            nc.vector.memset(vk[:, :, HD : 2 * HD], 1.0)

        # ---- PV + normalize emitters (operate on the PREVIOUS body's
        # exp tiles, so the PE never waits on ScalarE) ----
        def pv_alloc():
            return [
                pv_psum.tile([P, QB], F32, tag="pv_a", name="pv_a"),
                pv_psum.tile([P, QB], F32, tag="pv_b", name="pv_b"),
            ]

        def pv_step(state, kt):
            pair, qb, pv, ex_list = state
            for h2 in (0, 1):
                head = 2 * pair + h2
                nc.tensor.matmul(
                    pv[h2][:],
                    lhsT=v_t[kt][:, head * 2 * HD : (head + 1) * 2 * HD],
                    rhs=ex_list[kt][:, h2 * QB : (h2 + 1) * QB],
                    start=(kt == 0),
                    stop=(kt == N_KT - 1),
                )

        def norm(state):
            pair, qb, pv, ex_list = state
            qsl = slice(qb * QB, (qb + 1) * QB)
            for h2 in (0, 1):
                cpy = nrm_pool.tile([HD, QB], F32, tag="cpy", name="cpy")
                cpd = nrm_pool.tile([HD, QB], F32, tag="cpd", name="cpd")
                nc.vector.tensor_copy(cpy[:], pv[h2][0:HD, :])
                nc.vector.tensor_copy(cpd[:], pv[h2][HD:P, :])
                dst = yT_t[pair][h2 * HD : (h2 + 1) * HD, qsl]
                rc = nrm_pool.tile([HD, QB], F32, tag="rc", name="rc")
                # approx_fast is ~18-bit accurate; only valid with
                # base-partition-0 operands (breaks at other bases on HW)
                nc.vector.reciprocal_approx_fast(rc[:], cpd[:])
                nc.vector.tensor_mul(dst, cpy[:], rc[:])

        # ---- output projection for one token tile (128 rows) ----
        def oproj_mt(mt, evict_engine=None):
            msl = slice(mt * P, (mt + 1) * P)
            ob = ob_pool.tile([P, D], F32, tag="ob", name="ob")
            for nb in range(D // QB):
                op = pp_psum.tile([P, QB], F32, tag="pp", name="op")
                for ktile in range(DG // P):
                    nc.tensor.matmul(
                        op[:],
                        lhsT=yT_t[ktile][:, msl],
                        rhs=wo_t[:, ktile * D + nb * QB : ktile * D + (nb + 1) * QB],
                        start=(ktile == 0),
                        stop=(ktile == DG // P - 1),
                    )
                # tail tiles split evictions across ScalarE/VectorE
                if evict_engine and nb % 2 == 0:
                    nc.scalar.copy(ob[:, nb * QB : (nb + 1) * QB], op[:])
                else:
                    nc.vector.tensor_copy(ob[:, nb * QB : (nb + 1) * QB], op[:])
            nc.sync.dma_start(t["partial"][msl, :], ob[:])

        # ---- attention body: emits SC+exp for (pair, qb) while draining
        # the previous body's PV accumulation (2 steps/chunk over chunks
        # 0-7), its normalize at chunk 8, and filler work. Returns a state
        # tuple the next body uses to finish this iteration. ----
        def body(pair, qb, prev_state, fillers={}, self_pv=False, drain_rate=2):
            kt_p = kT_t[pair]
            qt_p = qT_t[pair]
            qsl = slice(qb * QB, (qb + 1) * QB)
            ex_list = []
            pv = None
            pv_self = [None]
            for kt in range(N_KT):
                ksl = slice(kt * P, (kt + 1) * P)
                sc = sc_psum.tile([P, 2 * QB], F32, tag="sc", name="sc")
                # head-pair scoresT matmuls (K=64) row-pack onto disjoint
                # PE row groups and run concurrently
                nc.tensor.matmul(
                    sc[:, 0:QB],
                    lhsT=kt_p[0:HD, ksl],
                    rhs=qt_p[0:HD, qsl],
                    start=True,
                    stop=True,
                )
                nc.tensor.matmul(
                    sc[:, QB : 2 * QB],
                    lhsT=kt_p[HD:P, ksl],
                    rhs=qt_p[HD:P, qsl],
                    start=True,
                    stop=True,
                )
                ex = expT_pool.tile([P, 2 * QB], BF16, tag="ex", name="ex")
                nc.scalar.activation(ex[:], sc[:], Exp, scale=SCALE)
                ex_list.append(ex)
                if prev_state is not None:
                    if pv is None:
                        pv = pv_alloc()
                        prev_state = (prev_state[0], prev_state[1], pv, prev_state[2])
                    if drain_rate == 1:
                        pv_step(prev_state, kt)
                    elif kt < 8:
                        pv_step(prev_state, 2 * kt)
                        pv_step(prev_state, 2 * kt + 1)
                    elif kt == 8:
                        norm(prev_state)
                    elif self_pv and kt >= 9:
                        # last body: start own PV on freed slots (chunks 9-15)
                        if pv_self[0] is None:
                            pv_self[0] = pv_alloc()
                        st = (pair, qb, pv_self[0], ex_list)
                        pv_step(st, 2 * (kt - 9))
                        pv_step(st, 2 * (kt - 9) + 1)
                for f in fillers.get(kt, ()):
                    f()
            if prev_state is not None and drain_rate == 1:
                norm(prev_state)
            if self_pv:
                st = (pair, qb, pv_self[0], ex_list)
                pv_step(st, 14)
                pv_step(st, 15)
                norm(st)
                return None
            return (pair, qb, ex_list)

        def drain(prev_state):
            if prev_state is None:
                return
            pv = pv_alloc()
            st = (prev_state[0], prev_state[1], pv, prev_state[2])
            for kt in range(N_KT):
                pv_step(st, kt)
            norm(st)

        # ---- schedule ----
        G = lambda *a: (lambda: qk_group(*a))
        V = lambda kt: (lambda: v_group(kt))
        O = lambda mt, eng=None: (lambda: oproj_mt(mt, eng))

        qk_group(wk_t, bk_t, kT_t, 0, 0)
        qk_group(wq_t, bq_t, qT_t, 0, 0)
        s = body(0, 0, None, fillers={
            2: [G(wk_t, bk_t, kT_t, 0, 1)],
            5: [G(wk_t, bk_t, kT_t, 0, 2)],
            8: [G(wk_t, bk_t, kT_t, 0, 3)],
            3: [V(0)], 4: [V(1)], 6: [V(2)], 7: [V(3)],
            9: [V(4)], 10: [V(5)], 11: [V(6)], 12: [V(7)],
            13: [G(wq_t, bq_t, qT_t, 0, 1)],
            14: [G(wq_t, bq_t, qT_t, 0, 2)],
            15: [G(wq_t, bq_t, qT_t, 0, 3)],
        })
        s = body(0, 1, s, drain_rate=1, fillers={
            **{kt: [V(kt + 8)] for kt in range(8)},
            9: [G(wk_t, bk_t, kT_t, 1, 0)],
            12: [G(wk_t, bk_t, kT_t, 1, 1)],
        })
        s = body(0, 2, s, fillers={
            9: [G(wk_t, bk_t, kT_t, 1, 2)],
            11: [G(wk_t, bk_t, kT_t, 1, 3)],
            13: [G(wq_t, bq_t, qT_t, 1, 0)],
        })
        s = body(0, 3, s, fillers={
            9: [G(wq_t, bq_t, qT_t, 1, 1)],
            11: [G(wq_t, bq_t, qT_t, 1, 2)],
            13: [G(wq_t, bq_t, qT_t, 1, 3)],
        })
        s = body(1, 0, s)
        s = body(1, 1, s, fillers={
            9: [O(0)], 11: [O(1)], 13: [O(2)], 15: [O(3)],
        })
        s = body(1, 2, s, fillers={
            9: [O(4)], 11: [O(5)], 13: [O(6)], 15: [O(7)],
        })
        s = body(1, 3, s, self_pv=True, fillers={
            9: [O(8)], 11: [O(9)], 13: [O(10)], 15: [O(11)],
        })
        drain(s)
        for mt in range(12, 16):
            oproj_mt(mt, nc.scalar)


def _build():
    import concourse.bacc as bacc
    import concourse.tile as tile
    from concourse import mybir

    F32 = mybir.dt.float32
    BF16 = mybir.dt.bfloat16

    nc = bacc.Bacc(
        "TRN2", target_bir_lowering=False, debug=False, num_devices=8
    )
    t = {
        "xq": nc.dram_tensor(
            "xq", (N_QB, P, N_DT * QB), BF16, kind="ExternalInput"
        ).ap(),
        "wqT": nc.dram_tensor("wqT", (P, N_DT * DG), BF16, kind="ExternalInput").ap(),
        "wkT": nc.dram_tensor("wkT", (P, N_DT * DG), BF16, kind="ExternalInput").ap(),
        "wvT": nc.dram_tensor("wvT", (P, N_DT * DG), BF16, kind="ExternalInput").ap(),
        "woT": nc.dram_tensor("woT", (P, (DG // P) * D), BF16, kind="ExternalInput").ap(),
        "bq2": nc.dram_tensor(
            "bq2", (P, DG // P), F32, kind="ExternalInput"
        ).ap(),
        "bk2": nc.dram_tensor(
            "bk2", (P, DG // P), F32, kind="ExternalInput"
        ).ap(),
        "bvb": nc.dram_tensor("bvb", (P, DG), F32, kind="ExternalInput").ap(),
        "partial": nc.dram_tensor(
            "partial", (NTOK, D), F32, kind="ExternalOutput"
        ).ap(),
    }
    with tile.TileContext(nc) as tc:
        _emit(tc, t)
    nc.compile()
    return nc


_CACHE = {}


def _get_nc():
    if "nc" not in _CACHE:
        _CACHE["nc"] = _build()
    return _CACHE["nc"]


def make_in_maps(x, Wq, bq, Wk, bk, Wv, bv, Wo):
    """Per-core host-side sharding: core c -> batch c//4, head group c%4."""

    def pack(wT, a):
        # (a*P, d) -> (P, a*d): row p holds the a contraction-tile slices
        # back to back, matching the packed SBUF tile layout
        d = wT.shape[1]
        return np.ascontiguousarray(
            wT.reshape(a, P, d).transpose(1, 0, 2).reshape(P, a * d)
        ).astype(_BF16)

    in_maps = []
    for c in range(8):
        b, g = divmod(c, 4)
        sl = slice(DG * g, DG * (g + 1))
        xT = x[b].T  # (D, NTOK)
        xq = np.ascontiguousarray(
            xT.reshape(N_DT, P, N_QB, QB).transpose(2, 1, 0, 3).reshape(
                N_QB, P, N_DT * QB
            )
        ).astype(_BF16)
        in_maps.append(
            {
                "xq": xq,
                "wqT": pack(Wq[sl].T, N_DT),
                "wkT": pack(Wk[sl].T, N_DT),
                "wvT": pack(Wv[sl].T, N_DT),
                "woT": pack(Wo[:, sl].T, DG // P),
                "bq2": np.ascontiguousarray(
                    bq[sl].reshape(DG // P, P).T
                ).astype(np.float32),
                "bk2": np.ascontiguousarray(
                    bk[sl].reshape(DG // P, P).T
                ).astype(np.float32),
                "bvb": np.ascontiguousarray(
                    np.broadcast_to(bv[sl][None, :], (P, DG))
                ).astype(np.float32),
            }
        )
    return in_maps


def kernel(x, Wq, bq, Wk, bk, Wv, bv, Wo, bo, _spmd_kwargs=None):
    from concourse.bass_utils import run_bass_kernel_spmd

    x, Wq, bq, Wk, bk, Wv, bv, Wo, bo = (
        np.asarray(a, np.float32) for a in (x, Wq, bq, Wk, bk, Wv, bv, Wo, bo)
    )
    nc = _get_nc()
    in_maps = make_in_maps(x, Wq, bq, Wk, bk, Wv, bv, Wo)
    res = run_bass_kernel_spmd(
        nc, in_maps, list(range(8)), **(_spmd_kwargs or {})
    )
    _CACHE["last_results"] = res
    out = np.empty((B, NTOK, D), np.float32)
    for b in range(B):
        acc = res.results[4 * b]["partial"].astype(np.float32).copy()
        for g in range(1, 4):
            acc += res.results[4 * b + g]["partial"]
        out[b] = acc + bo[None, :]
    return out


# revision 28
# speedup vs baseline: 1.0223x; 1.0115x over previous
"""Trainium2 Bass kernel for 16-head MHA (B=2, N=2048, D=1024, H=16).

Sharding: 8 cores = batch(2) x head-group(4). Each core computes 4 heads of
one batch element end-to-end (QKV projections, attention, and its partial
contribution to the output projection). The output projection is a sum over
head features, so each core returns a (N, D) partial product; the host sums
the 4 partials per batch and adds the output bias during unshard.

Per-core dataflow (all matmuls bf16 inputs, f32 PSUM accumulation):
  qT/kT = W @ x.T          (head-feature-major layout, 2 tiles of (128, N))
  v     = x @ Wv.T + bv    stored per key-tile as [v_h | ones] blocks
  scoresT[kt] = k @ q.T    (keys on partitions -> softmax denom comes from PE)
  expT = exp(SCALE*scoresT)  on ScalarE, reading PSUM directly
  [yT; denom] += [v|1].T @ expT  accumulated over key tiles
  yT_norm = yT * 1/denom   (denominator broadcast across partitions by PE)
  partial = yT_norm.T @ WoT
Heads are processed in pairs occupying partitions 0-63 / 64-127 so the two
scoresT matmuls (contraction K=64) row-pack onto disjoint PE row groups.
"""

import sys
from contextlib import ExitStack

import numpy as np

if "/opt/trn_rl_repo" not in sys.path:
    sys.path.insert(0, "/opt/trn_rl_repo")

import ml_dtypes

P = 128
B = 2
NTOK = 2048  # sequence length
D = 1024  # model dim
H_PER_CORE = 4  # heads per core
HD = 64  # head dim
DG = H_PER_CORE * HD  # head-group feature width per core (256)
QB = 512  # query block (matmul free dim)
N_QB = NTOK // QB  # 4
N_KT = NTOK // P  # 16 key tiles
N_DT = D // P  # 8 contraction tiles for projections
SCALE = HD ** -0.5

_BF16 = ml_dtypes.bfloat16


def _emit(tc, t):
    import concourse.bass as bass
    from concourse import mybir

    F32 = mybir.dt.float32
    BF16 = mybir.dt.bfloat16
    Exp = mybir.ActivationFunctionType.Exp
    nc = tc.nc

    with ExitStack() as ctx:
        consts = ctx.enter_context(tc.tile_pool(name="consts", bufs=1))
        # PSUM budget (8 banks): sc 2x2 + pv 2x1 + pp 2x1 = 8
        pp_psum = ctx.enter_context(
            tc.tile_pool(name="pp_psum", bufs=2, space="PSUM")
        )
        sc_psum = ctx.enter_context(
            tc.tile_pool(name="sc_psum", bufs=2, space="PSUM")
        )
        pv_psum = ctx.enter_context(
            tc.tile_pool(name="pv_psum", bufs=1, space="PSUM")
        )
        expT_pool = ctx.enter_context(tc.tile_pool(name="expT", bufs=18))
        nrm_pool = ctx.enter_context(tc.tile_pool(name="nrm", bufs=4))
        ob_pool = ctx.enter_context(tc.tile_pool(name="ob", bufs=4))

        # ---- resident SBUF tensors (single tiles, one DMA each) ----
        xT_t = consts.tile([P, N_DT * NTOK], BF16, tag="xT", name="xT")
        wq_t = consts.tile([P, N_DT * DG], BF16, tag="wq", name="wq")
        wk_t = consts.tile([P, N_DT * DG], BF16, tag="wk", name="wk")
        wv_t = consts.tile([P, N_DT * DG], BF16, tag="wv", name="wv")
        wo_t = consts.tile([P, (DG // P) * D], BF16, tag="wo", name="wo")
        bq_t = consts.tile([P, DG // P], F32, tag="bq", name="bq")
        bk_t = consts.tile([P, DG // P], F32, tag="bk", name="bk")
        bvb_t = consts.tile([P, DG], F32, tag="bvb", name="bvb")
        qT_t = [consts.tile([P, NTOK], BF16, tag=f"qT{i}", name=f"qT{i}") for i in range(DG // P)]
        kT_t = [consts.tile([P, NTOK], BF16, tag=f"kT{i}", name=f"kT{i}") for i in range(DG // P)]
        # v per key tile: 4 head blocks of [v_h (64 cols) | ones (64 cols)]
        v_t = [
            consts.tile([P, H_PER_CORE * 2 * HD], BF16, tag=f"v{i}", name=f"v{i}")
            for i in range(N_KT)
        ]
        yT_t = [consts.tile([P, NTOK], BF16, tag=f"yT{i}", name=f"yT{i}") for i in range(DG // P)]

        # ---- ACT exp-table warmup (hides ~2.7us ACT_TABLE_LOAD) ----
        warm = nrm_pool.tile([P, 32], F32, tag="warm", name="warm", bufs=1)
        nc.vector.memset(warm[:], 0.0)
        nc.scalar.activation(warm[:, 0:16], warm[:, 16:32], Exp)

        # ---- input DMAs: host pre-packs every tensor into its exact SBUF
        # layout, so each load is a fully contiguous 2D burst ----
        xT_v = xT_t.rearrange("p (a n) -> p a n", a=N_DT)

        def dma_x(qb):
            qsl = slice(qb * QB, (qb + 1) * QB)
            nc.sync.dma_start(
                xT_v[:, :, qsl],
                t["xq"][qb].rearrange("p (a n) -> p a n", a=N_DT),
            )

        nc.sync.dma_start(wk_t[:], t["wkT"][:])
        dma_x(0)
        nc.sync.dma_start(wq_t[:], t["wqT"][:])
        nc.sync.dma_start(bk_t[:], t["bk2"][:])
        nc.sync.dma_start(bq_t[:], t["bq2"][:])
        dma_x(1)
        nc.sync.dma_start(wv_t[:], t["wvT"][:])
        nc.sync.dma_start(bvb_t[:], t["bvb"][:])
        dma_x(2)
        dma_x(3)
        nc.sync.dma_start(wo_t[:], t["woT"][:])

        # ---- PE HAM warmup: dummy matmuls fill the DMA wait so the real
        # projections start at the 2.4GHz clock ----
        wmm = nrm_pool.tile([P, 16], BF16, tag="wmm", name="wmm", bufs=1)
        nc.vector.memset(wmm[:], 0.5)
        wps = pp_psum.tile([P, 16], F32, tag="pp", name="wps")
        for _ in range(24):
            nc.tensor.matmul(
                wps[0:16, :], lhsT=wmm[:, 0:16], rhs=wmm[:], start=True, stop=True
            )
        nc.vector.tensor_copy(warm[0:16, 16:32], wps[0:16, :])


        # slice helpers into the packed single tiles
        def w_sl(wt, dt, pt):
            o = dt * DG + pt * P
            return wt[:, o : o + P]

        def x_sl(dt, lo, hi):
            return xT_t[:, dt * NTOK + lo : dt * NTOK + hi]

        # ---- projection group emitters (each: 8 matmuls + eviction) ----
        def qk_group(wt, bt, dst, pt, qb):
            pp = pp_psum.tile([P, QB], F32, tag="pp", name="pp")
            for dt in range(N_DT):
                nc.tensor.matmul(
                    pp[:],
                    lhsT=w_sl(wt, dt, pt),
                    rhs=x_sl(dt, qb * QB, (qb + 1) * QB),
                    start=(dt == 0),
                    stop=(dt == N_DT - 1),
                )
            nc.vector.tensor_scalar_add(
                dst[pt][:, qb * QB : (qb + 1) * QB], pp[:], bt[:, pt : pt + 1]
            )

        def v_group(kt):
            pp = pp_psum.tile([P, QB], F32, tag="pp", name="pp_v")
            for dt in range(N_DT):
                nc.tensor.matmul(
                    pp[:, 0:DG],
                    lhsT=x_sl(dt, kt * P, (kt + 1) * P),
                    rhs=wv_t[:, dt * DG : (dt + 1) * DG],
                    start=(dt == 0),
                    stop=(dt == N_DT - 1),
                )
            vk = v_t[kt].rearrange("p (h w) -> p h w", h=H_PER_CORE)
            nc.vector.tensor_add(
                vk[:, :, 0:HD],
                pp[:, 0:DG].rearrange("p (h w) -> p h w", h=H_PER_CORE),
                bvb_t[:].rearrange("p (h w) -> p h w", h=H_PER_CORE),
            )
            nc.vector.memset(vk[:, :, HD : 2 * HD], 1.0)

        # ---- PV + normalize emitters (operate on the PREVIOUS body's
        # exp tiles, so the PE never waits on ScalarE) ----
        def pv_alloc():
            return [
                pv_psum.tile([P, QB], F32, tag="pv_a", name="pv_a"),
                pv_psum.tile([P, QB], F32, tag="pv_b", name="pv_b"),
            ]

        def pv_step(state, kt):
            pair, qb, pv, ex_list = state
            for h2 in (0, 1):
                head = 2 * pair + h2
                nc.tensor.matmul(
                    pv[h2][:],
                    lhsT=v_t[kt][:, head * 2 * HD : (head + 1) * 2 * HD],
                    rhs=ex_list[kt][:, h2 * QB : (h2 + 1) * QB],
                    start=(kt == 0),
                    stop=(kt == N_KT - 1),
                )

        def norm(state, final=False):
            pair, qb, pv, ex_list = state
            qsl = slice(qb * QB, (qb + 1) * QB)
            for h2 in (0, 1):
                cpd = nrm_pool.tile([HD, QB], F32, tag="cpd", name="cpd")
                if final:
                    # tail: no one needs the PSUM slots, so skip the y-copy
                    # (mult reads PSUM directly) and put one denominator
                    # copy on the otherwise-idle ScalarE
                    if h2 == 0:
                        nc.scalar.copy(cpd[:], pv[h2][HD:P, :])
                    else:
                        nc.vector.tensor_copy(cpd[:], pv[h2][HD:P, :])
                    cpy_ap = pv[h2][0:HD, :]
                else:
                    cpy = nrm_pool.tile([HD, QB], F32, tag="cpy", name="cpy")
                    nc.vector.tensor_copy(cpy[:], pv[h2][0:HD, :])
                    nc.vector.tensor_copy(cpd[:], pv[h2][HD:P, :])
                    cpy_ap = cpy[:]
                dst = yT_t[pair][h2 * HD : (h2 + 1) * HD, qsl]
                rc = nrm_pool.tile([HD, QB], F32, tag="rc", name="rc")
                # approx_fast is ~18-bit accurate; only valid with
                # base-partition-0 operands (breaks at other bases on HW)
                nc.vector.reciprocal_approx_fast(rc[:], cpd[:])
                nc.vector.tensor_mul(dst, cpy_ap, rc[:])

        # ---- output projection for one token tile (128 rows) ----
        def oproj_mt(mt, evict_engine=None):
            msl = slice(mt * P, (mt + 1) * P)
            ob = ob_pool.tile([P, D], F32, tag="ob", name="ob")
            for nb in range(D // QB):
                op = pp_psum.tile([P, QB], F32, tag="pp", name="op")
                for ktile in range(DG // P):
                    nc.tensor.matmul(
                        op[:],
                        lhsT=yT_t[ktile][:, msl],
                        rhs=wo_t[:, ktile * D + nb * QB : ktile * D + (nb + 1) * QB],
                        start=(ktile == 0),
                        stop=(ktile == DG // P - 1),
                    )
                # tail tiles split evictions across ScalarE/VectorE
                if evict_engine and nb % 2 == 0:
                    nc.scalar.copy(ob[:, nb * QB : (nb + 1) * QB], op[:])
                else:
                    nc.vector.tensor_copy(ob[:, nb * QB : (nb + 1) * QB], op[:])
            nc.sync.dma_start(t["partial"][msl, :], ob[:])

        # ---- attention body: emits SC+exp for (pair, qb) while draining
        # the previous body's PV accumulation (2 steps/chunk over chunks
        # 0-7), its normalize at chunk 8, and filler work. Returns a state
        # tuple the next body uses to finish this iteration. ----
        def body(pair, qb, prev_state, fillers={}, self_pv=False, drain_rate=2):
            kt_p = kT_t[pair]
            qt_p = qT_t[pair]
            qsl = slice(qb * QB, (qb + 1) * QB)
            ex_list = []
            pv = None
            pv_self = [None]
            for kt in range(N_KT):
                ksl = slice(kt * P, (kt + 1) * P)
                sc = sc_psum.tile([P, 2 * QB], F32, tag="sc", name="sc")
                # head-pair scoresT matmuls (K=64) row-pack onto disjoint
                # PE row groups and run concurrently
                nc.tensor.matmul(
                    sc[:, 0:QB],
                    lhsT=kt_p[0:HD, ksl],
                    rhs=qt_p[0:HD, qsl],
                    start=True,
                    stop=True,
                )
                nc.tensor.matmul(
                    sc[:, QB : 2 * QB],
                    lhsT=kt_p[HD:P, ksl],
                    rhs=qt_p[HD:P, qsl],
                    start=True,
                    stop=True,
                )
                ex = expT_pool.tile([P, 2 * QB], BF16, tag="ex", name="ex")
                nc.scalar.activation(ex[:], sc[:], Exp, scale=SCALE)
                ex_list.append(ex)
                if prev_state is not None:
                    if pv is None:
                        pv = pv_alloc()
                        prev_state = (prev_state[0], prev_state[1], pv, prev_state[2])
                    if drain_rate == 1:
                        pv_step(prev_state, kt)
                    elif kt < 8:
                        pv_step(prev_state, 2 * kt)
                        pv_step(prev_state, 2 * kt + 1)
                    elif kt == 8:
                        norm(prev_state)
                    elif self_pv and kt >= 9:
                        # last body: start own PV on freed slots (chunks 9-15)
                        if pv_self[0] is None:
                            pv_self[0] = pv_alloc()
                        st = (pair, qb, pv_self[0], ex_list)
                        pv_step(st, 2 * (kt - 9))
                        pv_step(st, 2 * (kt - 9) + 1)
                for f in fillers.get(kt, ()):
                    f()
            if prev_state is not None and drain_rate == 1:
                norm(prev_state)
            if self_pv:
                st = (pair, qb, pv_self[0], ex_list)
                pv_step(st, 14)
                pv_step(st, 15)
                norm(st, final=True)
                return None
            return (pair, qb, ex_list)

        def drain(prev_state):
            if prev_state is None:
                return
            pv = pv_alloc()
            st = (prev_state[0], prev_state[1], pv, prev_state[2])
            for kt in range(N_KT):
                pv_step(st, kt)
            norm(st)

        # ---- schedule ----
        G = lambda *a: (lambda: qk_group(*a))
        V = lambda kt: (lambda: v_group(kt))
        O = lambda mt, eng=None: (lambda: oproj_mt(mt, eng))

        qk_group(wk_t, bk_t, kT_t, 0, 0)
        qk_group(wq_t, bq_t, qT_t, 0, 0)
        s = body(0, 0, None, fillers={
            2: [G(wk_t, bk_t, kT_t, 0, 1)],
            5: [G(wk_t, bk_t, kT_t, 0, 2)],
            8: [G(wk_t, bk_t, kT_t, 0, 3)],
            3: [V(0)], 4: [V(1)], 6: [V(2)], 7: [V(3)],
            9: [V(4)], 10: [V(5)], 11: [V(6)], 12: [V(7)],
            13: [G(wq_t, bq_t, qT_t, 0, 1)],
            14: [G(wq_t, bq_t, qT_t, 0, 2)],
            15: [G(wq_t, bq_t, qT_t, 0, 3)],
        })
        s = body(0, 1, s, drain_rate=1, fillers={
            **{kt: [V(kt + 8)] for kt in range(8)},
            9: [G(wk_t, bk_t, kT_t, 1, 0)],
            12: [G(wk_t, bk_t, kT_t, 1, 1)],
        })
        s = body(0, 2, s, fillers={
            9: [G(wk_t, bk_t, kT_t, 1, 2)],
            11: [G(wk_t, bk_t, kT_t, 1, 3)],
            13: [G(wq_t, bq_t, qT_t, 1, 0)],
        })
        s = body(0, 3, s, fillers={
            9: [G(wq_t, bq_t, qT_t, 1, 1)],
            11: [G(wq_t, bq_t, qT_t, 1, 2)],
            13: [G(wq_t, bq_t, qT_t, 1, 3)],
        })
        s = body(1, 0, s)
        s = body(1, 1, s, fillers={
            9: [O(0)], 11: [O(1)], 13: [O(2)], 15: [O(3)],
        })
        s = body(1, 2, s, fillers={
            9: [O(4)], 11: [O(5)], 13: [O(6)], 15: [O(7)],
        })
        s = body(1, 3, s, self_pv=True, fillers={
            9: [O(8)], 11: [O(9)], 13: [O(10)], 15: [O(11)],
        })
        drain(s)
        for mt in range(12, 16):
            oproj_mt(mt, nc.scalar)


def _build():
    import concourse.bacc as bacc
    import concourse.tile as tile
    from concourse import mybir

    F32 = mybir.dt.float32
    BF16 = mybir.dt.bfloat16

    nc = bacc.Bacc(
        "TRN2", target_bir_lowering=False, debug=False, num_devices=8
    )
    t = {
        "xq": nc.dram_tensor(
            "xq", (N_QB, P, N_DT * QB), BF16, kind="ExternalInput"
        ).ap(),
        "wqT": nc.dram_tensor("wqT", (P, N_DT * DG), BF16, kind="ExternalInput").ap(),
        "wkT": nc.dram_tensor("wkT", (P, N_DT * DG), BF16, kind="ExternalInput").ap(),
        "wvT": nc.dram_tensor("wvT", (P, N_DT * DG), BF16, kind="ExternalInput").ap(),
        "woT": nc.dram_tensor("woT", (P, (DG // P) * D), BF16, kind="ExternalInput").ap(),
        "bq2": nc.dram_tensor(
            "bq2", (P, DG // P), F32, kind="ExternalInput"
        ).ap(),
        "bk2": nc.dram_tensor(
            "bk2", (P, DG // P), F32, kind="ExternalInput"
        ).ap(),
        "bvb": nc.dram_tensor("bvb", (P, DG), F32, kind="ExternalInput").ap(),
        "partial": nc.dram_tensor(
            "partial", (NTOK, D), F32, kind="ExternalOutput"
        ).ap(),
    }
    with tile.TileContext(nc) as tc:
        _emit(tc, t)
    nc.compile()
    return nc


_CACHE = {}


def _get_nc():
    if "nc" not in _CACHE:
        _CACHE["nc"] = _build()
    return _CACHE["nc"]


def make_in_maps(x, Wq, bq, Wk, bk, Wv, bv, Wo):
    """Per-core host-side sharding: core c -> batch c//4, head group c%4."""

    def pack(wT, a):
        # (a*P, d) -> (P, a*d): row p holds the a contraction-tile slices
        # back to back, matching the packed SBUF tile layout
        d = wT.shape[1]
        return np.ascontiguousarray(
            wT.reshape(a, P, d).transpose(1, 0, 2).reshape(P, a * d)
        ).astype(_BF16)

    in_maps = []
    for c in range(8):
        b, g = divmod(c, 4)
        sl = slice(DG * g, DG * (g + 1))
        xT = x[b].T  # (D, NTOK)
        xq = np.ascontiguousarray(
            xT.reshape(N_DT, P, N_QB, QB).transpose(2, 1, 0, 3).reshape(
                N_QB, P, N_DT * QB
            )
        ).astype(_BF16)
        in_maps.append(
            {
                "xq": xq,
                "wqT": pack(Wq[sl].T, N_DT),
                "wkT": pack(Wk[sl].T, N_DT),
                "wvT": pack(Wv[sl].T, N_DT),
                "woT": pack(Wo[:, sl].T, DG // P),
                "bq2": np.ascontiguousarray(
                    bq[sl].reshape(DG // P, P).T
                ).astype(np.float32),
                "bk2": np.ascontiguousarray(
                    bk[sl].reshape(DG // P, P).T
                ).astype(np.float32),
                "bvb": np.ascontiguousarray(
                    np.broadcast_to(bv[sl][None, :], (P, DG))
                ).astype(np.float32),
            }
        )
    return in_maps


def kernel(x, Wq, bq, Wk, bk, Wv, bv, Wo, bo, _spmd_kwargs=None):
    from concourse.bass_utils import run_bass_kernel_spmd

    x, Wq, bq, Wk, bk, Wv, bv, Wo, bo = (
        np.asarray(a, np.float32) for a in (x, Wq, bq, Wk, bk, Wv, bv, Wo, bo)
    )
    nc = _get_nc()
    in_maps = make_in_maps(x, Wq, bq, Wk, bk, Wv, bv, Wo)
    res = run_bass_kernel_spmd(
        nc, in_maps, list(range(8)), **(_spmd_kwargs or {})
    )
    _CACHE["last_results"] = res
    out = np.empty((B, NTOK, D), np.float32)
    for b in range(B):
        acc = res.results[4 * b]["partial"].astype(np.float32).copy()
        for g in range(1, 4):
            acc += res.results[4 * b + g]["partial"]
        out[b] = acc + bo[None, :]
    return out
